# revision 26
# baseline (speedup 1.0000x reference)
"""Trainium2 Bass kernel for nn_DynamicAdapter (dense-MoE adapter block).

Math (per reference):
  pooled = mean_s(hidden)                               [B, H]
  gate = softmax(MLP_sel(MLP_ana(pooled)))              [B, E]
  h1_e = gelu(x @ W1_e + b1_e)                          [T, H/2]
  eo_e = (h1_e @ W2_e + b2_e) * gate[b, e]              [T, H]
  fused = sum_e eo_e @ Wf_e + f_b + x                   [T, H]
  out = layernorm(fused) * ln_g + ln_b

Sharding: token-parallel. Core c handles tokens {(b, c*256+j)} — 1024 tokens.
Every core runs all 16 experts on its tokens (weights replicated), computes
the gate redundantly from a replicated bf16 copy of x, and writes its token
shard of the output. No collectives.

Layout: activations feature-major ([feat_part, token_free]) through mm1/mm2;
mm3 uses eo tiles as the stationary operand so the fused output comes out
token-major, which makes the residual+LayerNorm tail and output DMA natural.
All matmul operands bf16 (PSUM accumulation fp32); everything else fp32.
"""

import numpy as np
import ml_dtypes

import concourse.bacc as bacc
import concourse.mybir as mybir
import concourse.tile as tile
from concourse import bass_utils

BF16 = ml_dtypes.bfloat16

B, S, H, E = 4, 2048, 1024, 16
NCORES = 8
P = 128
TOK = B * S            # 8192 tokens total
TPC = TOK // NCORES    # 1024 tokens per core
SC = S // NCORES       # 256 tokens per (batch, core)
HT = H // P            # 8 h-tiles
F1 = H // 2            # 512 expert hidden
F1T = F1 // P          # 4 f1-tiles
TCH = 512              # token chunk for mm1/mm2 (2 batch-chunks)
NCH = TPC // TCH       # 2 chunks
TT = TPC // P          # 8 token-tiles for mm3/tail

dt16 = mybir.dt.bfloat16
dt32 = mybir.dt.float32
AF = mybir.ActivationFunctionType
ALU = mybir.AluOpType
AX = mybir.AxisListType

_BUILT = {}


def _build():
    if "nc" in _BUILT:
        return _BUILT["nc"]

    nc = bacc.Bacc("TRN2", target_bir_lowering=False, debug=False)

    # ---- kernel I/O ----
    xpool = nc.dram_tensor("xpool", [TOK, H], dt16, kind="ExternalInput").ap()
    xsh = nc.dram_tensor("xsh", [H, TPC], dt16, kind="ExternalInput").ap()
    xres = nc.dram_tensor("xres", [TPC, H], dt32, kind="ExternalInput").ap()
    w1 = nc.dram_tensor("w1", [E, HT, P, F1], dt16, kind="ExternalInput").ap()
    w2 = nc.dram_tensor("w2", [E, F1T, P, H], dt16, kind="ExternalInput").ap()
    wf = nc.dram_tensor("wf", [E, HT, P, H], dt16, kind="ExternalInput").ap()
    b1d = nc.dram_tensor("b1d", [P, E * F1T], dt32, kind="ExternalInput").ap()
    e2bTd = nc.dram_tensor("e2bTd", [P, HT * E], dt32, kind="ExternalInput").ap()
    a1 = nc.dram_tensor("a1", [HT, P, F1], dt16, kind="ExternalInput").ap()
    a2 = nc.dram_tensor("a2", [4, P, 256], dt16, kind="ExternalInput").ap()
    a3 = nc.dram_tensor("a3", [2, P, 128], dt16, kind="ExternalInput").ap()
    s1 = nc.dram_tensor("s1", [P, 64], dt16, kind="ExternalInput").ap()
    s2 = nc.dram_tensor("s2", [64, 32], dt16, kind="ExternalInput").ap()
    s3 = nc.dram_tensor("s3", [32, 16], dt16, kind="ExternalInput").ap()
    ab1 = nc.dram_tensor("ab1", [P, 4], dt32, kind="ExternalInput").ap()
    ab2 = nc.dram_tensor("ab2", [P, 2], dt32, kind="ExternalInput").ap()
    ab3 = nc.dram_tensor("ab3", [P, 1], dt32, kind="ExternalInput").ap()
    sb1 = nc.dram_tensor("sb1", [64, 1], dt32, kind="ExternalInput").ap()
    sb2 = nc.dram_tensor("sb2", [32, 1], dt32, kind="ExternalInput").ap()
    sb3 = nc.dram_tensor("sb3", [B, E], dt32, kind="ExternalInput").ap()
    ident4 = nc.dram_tensor("ident4", [4, 4], dt32, kind="ExternalInput").ap()
    fbbc_d = nc.dram_tensor("fbbc_d", [P, H], dt32, kind="ExternalInput").ap()
    gbc_d = nc.dram_tensor("gbc_d", [P, H], dt32, kind="ExternalInput").ap()
    bbc_d = nc.dram_tensor("bbc_d", [P, H], dt32, kind="ExternalInput").ap()
    out = nc.dram_tensor("out", [TPC, H], dt32, kind="ExternalOutput").ap()

    with tile.TileContext(nc) as tc:
        _emit(tc, locals())
    nc.compile()
    _BUILT["nc"] = nc
    return nc


def _emit(tc, t):
    nc = tc.nc
    with (
        tc.tile_pool(name="persist", bufs=1) as pp,
        tc.tile_pool(name="wpool", bufs=2) as wp,
        tc.tile_pool(name="hpool", bufs=2) as hp,
        tc.tile_pool(name="eopool", bufs=1) as ep,
        tc.tile_pool(name="ps1p", bufs=2, space="PSUM") as ps1p,
        tc.tile_pool(name="ps2p", bufs=2, space="PSUM") as ps2p,
    ):
        # ---------- critical-path DMAs first: x shard + expert-0 weights ----------
        xs = []
        for i in range(HT):
            xt = pp.tile([P, TPC], dt16, name=f"xs{i}", tag=f"xs{i}")
            nc.sync.dma_start(out=xt[:, :], in_=t["xsh"][i * P : (i + 1) * P, :])
            xs.append(xt)

        def fetch_weights(e):
            w1t = wp.tile([P, HT, F1], dt16, name=f"w1t{e}", tag="w1t")
            src1 = t["w1"][e].rearrange("i p f -> p i f")
            nc.sync.dma_start(out=w1t[:, 0:4, :], in_=src1[:, 0:4, :])
            nc.sync.dma_start(out=w1t[:, 4:8, :], in_=src1[:, 4:8, :])
            w2t = wp.tile([P, F1T, H], dt16, name=f"w2t{e}", tag="w2t")
            src2 = t["w2"][e].rearrange("m p h -> p m h")
            nc.sync.dma_start(out=w2t[:, 0:2, :], in_=src2[:, 0:2, :])
            nc.sync.dma_start(out=w2t[:, 2:4, :], in_=src2[:, 2:4, :])
            wft = wp.tile([P, HT, H], dt16, name=f"wft{e}", tag="wft")
            srcf = t["wf"][e].rearrange("i p g -> p i g")
            nc.sync.dma_start(out=wft[:, 0:2, :], in_=srcf[:, 0:2, :])
            nc.sync.dma_start(out=wft[:, 2:4, :], in_=srcf[:, 2:4, :])
            nc.sync.dma_start(out=wft[:, 4:6, :], in_=srcf[:, 4:6, :])
            nc.sync.dma_start(out=wft[:, 6:8, :], in_=srcf[:, 6:8, :])
            return w1t, w2t, wft

        w_cache = {0: fetch_weights(0)}
        b1_sb = pp.tile([P, E * F1T], dt32, name="b1_sb", tag="b1_sb")
        nc.sync.dma_start(out=b1_sb[:, :], in_=t["b1d"][:, :])
        e2bT_sb = pp.tile([P, HT * E], dt32, name="e2bT_sb", tag="e2bT_sb")
        nc.sync.dma_start(out=e2bT_sb[:, :], in_=t["e2bTd"][:, :])
        id4 = pp.tile([4, 4], dt32, name="id4", tag="id4")
        nc.sync.dma_start(out=id4[:, :], in_=t["ident4"][:, :])
        eps = pp.tile([P, 1], dt32, name="eps", tag="eps")
        nc.vector.memset(eps[:, :], 1e-5)

        fused = []
        for tau in range(TT):
            ft = pp.tile([P, H], dt32, name=f"fused{tau}", tag=f"fused{tau}")
            fused.append(ft)

        # per-(b,e) gate broadcast to all partitions, and gate-scaled e2 bias
        gate_bc = pp.tile([P, B * E], dt32, name="gate_bc", tag="gate_bc")
        e2bs = pp.tile([P, HT * B * E], dt32, name="e2bs", tag="e2bs")

        # batch-indicator stationaries for pooling: column b holds 1/S
        ind = pp.tile([P, B], dt16, name="ind", tag="ind")
        nc.vector.memset(ind[:, :], 0.0)
        for b in range(B):
            nc.vector.memset(ind[:, b : b + 1], 1.0 / S)

        # ---------- phase 0: pooling + gate ----------
        with (
            tc.tile_pool(name="ph0", bufs=6) as xp,
            tc.tile_pool(name="gw", bufs=1) as gw,
            tc.tile_pool(name="ps0p", bufs=2, space="PSUM") as ps0p,
            tc.tile_pool(name="psgp", bufs=2, space="PSUM") as psgp,
        ):
            # gate weights first: small, and needed as soon as pooling lands
            a1_sb = gw.tile([P, HT, F1], dt16, name="a1_sb", tag="a1_sb")
            nc.sync.dma_start(out=a1_sb[:, :, :], in_=t["a1"].rearrange("i p f -> p i f"))
            a2_sb = gw.tile([P, 4, 256], dt16, name="a2_sb", tag="a2_sb")
            nc.sync.dma_start(out=a2_sb[:, :, :], in_=t["a2"].rearrange("i p f -> p i f"))
            a3_sb = gw.tile([P, 2, 128], dt16, name="a3_sb", tag="a3_sb")
            nc.sync.dma_start(out=a3_sb[:, :, :], in_=t["a3"].rearrange("i p f -> p i f"))
            s1_sb = gw.tile([P, 64], dt16, name="s1_sb", tag="s1_sb")
            nc.sync.dma_start(out=s1_sb[:, :], in_=t["s1"][:, :])
            s2_sb = gw.tile([64, 32], dt16, name="s2_sb", tag="s2_sb")
            nc.sync.dma_start(out=s2_sb[:, :], in_=t["s2"][:, :])
            s3_sb = gw.tile([32, 16], dt16, name="s3_sb", tag="s3_sb")
            nc.sync.dma_start(out=s3_sb[:, :], in_=t["s3"][:, :])
            ab1_sb = gw.tile([P, 4], dt32, name="ab1_sb", tag="ab1_sb")
            nc.sync.dma_start(out=ab1_sb[:, :], in_=t["ab1"][:, :])
            ab2_sb = gw.tile([P, 2], dt32, name="ab2_sb", tag="ab2_sb")
            nc.sync.dma_start(out=ab2_sb[:, :], in_=t["ab2"][:, :])
            ab3_sb = gw.tile([P, 1], dt32, name="ab3_sb", tag="ab3_sb")
            nc.sync.dma_start(out=ab3_sb[:, :], in_=t["ab3"][:, :])
            sb1_sb = gw.tile([64, 1], dt32, name="sb1_sb", tag="sb1_sb")
            nc.sync.dma_start(out=sb1_sb[:, :], in_=t["sb1"][:, :])
            sb2_sb = gw.tile([32, 1], dt32, name="sb2_sb", tag="sb2_sb")
            nc.sync.dma_start(out=sb2_sb[:, :], in_=t["sb2"][:, :])
            sb3_sb = gw.tile([B, E], dt32, name="sb3_sb", tag="sb3_sb")
            nc.sync.dma_start(out=sb3_sb[:, :], in_=t["sb3"][:, :])

            # mean-pool over the full sequence: indicator-column matmuls
            # accumulate all 64 token-tiles into one [B, 512] psum per h-half.
            stack = gw.tile([B, H], dt32, name="stack", tag="stack")
            for hc in range(2):
                psp = ps0p.tile([B, TCH], dt32, name="psp", tag="ps0")
                n_t = TOK // P
                for tt_ in range(n_t):
                    b = tt_ // (S // P)
                    xt = xp.tile([P, TCH], dt16, name="xt", tag="xt")
                    nc.sync.dma_start(
                        out=xt[:, :],
                        in_=t["xpool"][tt_ * P : (tt_ + 1) * P, hc * TCH : (hc + 1) * TCH],
                    )
                    nc.tensor.matmul(
                        psp[:, :], ind[:, :], xt[:, :],
                        start=(tt_ == 0), stop=(tt_ == n_t - 1),
                    )
                nc.scalar.copy(stack[:, hc * TCH : (hc + 1) * TCH], psp[:, :])

            # transpose pooled [B, H] -> 8 feature-major [P, B] bf16 tiles
            ptb = []
            for i in range(HT):
                pst = psgp.tile([P, B], dt32, name="pst", tag="psg")
                nc.tensor.transpose(pst[:, :], stack[:, i * P : (i + 1) * P], id4[:, :])
                pb = gw.tile([P, B], dt16, name=f"ptb{i}", tag=f"ptb{i}")
                nc.scalar.copy(pb[:, :], pst[:, :])
                ptb.append(pb)

            # gate MLP (feature-major)
            t1 = gw.tile([P, 16], dt16, name="t1", tag="t1")
            for m in range(4):
                psg = psgp.tile([P, B], dt32, name="psg1", tag="psg")
                for i in range(HT):
                    nc.tensor.matmul(
                        psg[:, :], a1_sb[:, i, m * P : (m + 1) * P], ptb[i][:, :],
                        start=(i == 0), stop=(i == HT - 1),
                    )
                nc.scalar.activation(
                    t1[:, m * B : (m + 1) * B], psg[:, :], AF.Gelu,
                    bias=ab1_sb[:, m : m + 1],
                )
            t2 = gw.tile([P, 8], dt16, name="t2", tag="t2")
            for m in range(2):
                psg = psgp.tile([P, B], dt32, name="psg2", tag="psg")
                for i in range(4):
                    nc.tensor.matmul(
                        psg[:, :], a2_sb[:, i, m * P : (m + 1) * P], t1[:, i * B : (i + 1) * B],
                        start=(i == 0), stop=(i == 3),
                    )
                nc.scalar.activation(
                    t2[:, m * B : (m + 1) * B], psg[:, :], AF.Gelu,
                    bias=ab2_sb[:, m : m + 1],
                )
            t3 = gw.tile([P, B], dt16, name="t3", tag="t3")
            psg = psgp.tile([P, B], dt32, name="psg3", tag="psg")
            for i in range(2):
                nc.tensor.matmul(
                    psg[:, :], a3_sb[:, i, :], t2[:, i * B : (i + 1) * B],
                    start=(i == 0), stop=(i == 1),
                )
            nc.scalar.activation(t3[:, :], psg[:, :], AF.Identity, bias=ab3_sb[:, 0:1])

            g1 = gw.tile([64, B], dt16, name="g1", tag="g1")
            psg = psgp.tile([64, B], dt32, name="psg4", tag="psg")
            nc.tensor.matmul(psg[:, :], s1_sb[:, :], t3[:, :], start=True, stop=True)
            nc.scalar.activation(g1[:, :], psg[:, :], AF.Gelu, bias=sb1_sb[:, 0:1])

            g2 = gw.tile([32, B], dt16, name="g2", tag="g2")
            psg = psgp.tile([32, B], dt32, name="psg5", tag="psg")
            nc.tensor.matmul(psg[:, :], s2_sb[:, :], g1[:, :], start=True, stop=True)
            nc.scalar.activation(g2[:, :], psg[:, :], AF.Gelu, bias=sb2_sb[:, 0:1])

            # flip to token-major: z[b, e]
            z = gw.tile([B, E], dt32, name="z", tag="z")
            psg = psgp.tile([B, E], dt32, name="psg6", tag="psg")
            nc.tensor.matmul(psg[:, :], g2[:, :], s3_sb[:, :], start=True, stop=True)
            nc.scalar.copy(z[:, :], psg[:, :])
            nc.vector.tensor_add(z[:, :], z[:, :], sb3_sb[:, :])

            # softmax over E (free dim)
            mx = gw.tile([B, 1], dt32, name="mx", tag="mx")
            nc.vector.reduce_max(mx[:, :], z[:, :], axis=AX.X)
            nc.vector.tensor_scalar_sub(z[:, :], z[:, :], mx[:, 0:1])
            sums = gw.tile([B, 1], dt32, name="sums", tag="sums")
            exps = gw.tile([B, E], dt32, name="exps", tag="exps")
            nc.scalar.activation(exps[:, :], z[:, :], AF.Exp, accum_out=sums[:, 0:1])
            rinv = gw.tile([B, 1], dt32, name="rinv", tag="rinv")
            nc.vector.reciprocal(rinv[:, :], sums[:, :])
            gate4 = gw.tile([B, E], dt32, name="gate4", tag="gate4")
            nc.vector.tensor_scalar_mul(gate4[:, :], exps[:, :], rinv[:, 0:1])

            # broadcast gate to all 128 partitions via DRAM bounce
            with tc.tile_pool(name="dramp", bufs=1, space="DRAM") as dp:
                gsc = dp.tile([1, B * E], dt32, name="gsc", tag="gsc")
                nc.sync.dma_start(
                    out=gsc.rearrange("o (b e) -> (o b) e", b=B), in_=gate4[:, :]
                )
                gflat = gw.tile([1, B * E], dt32, name="gflat", tag="gflat")
                nc.sync.dma_start(out=gflat[:, :], in_=gsc[:, :])
            nc.gpsimd.partition_broadcast(gate_bc[:, :], gflat[:, :])

            # gate-scaled e2 bias: e2bs[p, (i, b, e)] = e2bT[p, (i, e)] * gate[b, e]
            for i in range(HT):
                for b in range(B):
                    nc.vector.tensor_mul(
                        e2bs[:, i * (B * E) + b * E : i * (B * E) + (b + 1) * E],
                        e2bT_sb[:, i * E : (i + 1) * E],
                        gate_bc[:, b * E : (b + 1) * E],
                    )

        # ---------- tail constants (needed only at the end) ----------
        fbbc = pp.tile([P, H], dt32, name="fbbc", tag="fbbc")
        nc.sync.dma_start(out=fbbc[:, :], in_=t["fbbc_d"][:, :])
        gbc = pp.tile([P, H], dt32, name="gbc", tag="gbc")
        nc.sync.dma_start(out=gbc[:, :], in_=t["gbc_d"][:, :])
        bbc = pp.tile([P, H], dt32, name="bbc", tag="bbc")
        nc.sync.dma_start(out=bbc[:, :], in_=t["bbc_d"][:, :])

        # ---------- pools for the expert loop + interleaved tail ----------
        txf = tc.alloc_tile_pool(name="txf", bufs=5)
        ps3p = tc.alloc_tile_pool(name="ps3p", bufs=4, space="PSUM")
        tp = tc.alloc_tile_pool(name="tail", bufs=2)
        otp = tc.alloc_tile_pool(name="otp", bufs=2)
        sqp = tc.alloc_tile_pool(name="sqp", bufs=1)
        xrfs = {}

        def emit_xrf(tau):
            # residual + f_b prep; no expert-loop deps, runs on DVE slack
            xrf = txf.tile([P, H], dt32, name=f"xrf{tau}", tag="xrf")
            nc.sync.dma_start(out=xrf[:, :], in_=t["xres"][tau * P : (tau + 1) * P, :])
            nc.vector.tensor_add(xrf[:, :], xrf[:, :], fbbc[:, :])
            xrfs[tau] = xrf

        def emit_tail(tau):
            # layernorm tail for one token tile; DVE: +xrf, reduce, scale*g;
            # ACT: center, square-accum, sqrt; GpSimd: +ln_b.
            f2 = fused[tau]
            nc.vector.tensor_add(f2[:, :], f2[:, :], xrfs[tau][:, :])
            ssum = tp.tile([P, 1], dt32, name="ssum", tag="ssum")
            nc.vector.reduce_sum(ssum[:, :], f2[:, :], axis=AX.X)
            negmu = tp.tile([P, 1], dt32, name="negmu", tag="negmu")
            nc.vector.tensor_scalar_mul(negmu[:, :], ssum[:, :], -1.0 / H)
            nc.scalar.activation(f2[:, :], f2[:, :], AF.Identity, bias=negmu[:, 0:1])
            sq = sqp.tile([P, H], dt16, name="sq", tag="sq")
            ssq = tp.tile([P, 1], dt32, name="ssq", tag="ssq")
            nc.scalar.activation(sq[:, :], f2[:, :], AF.Square, accum_out=ssq[:, 0:1])
            stdv = tp.tile([P, 1], dt32, name="stdv", tag="stdv")
            nc.scalar.activation(
                stdv[:, :], ssq[:, :], AF.Sqrt, scale=1.0 / H, bias=eps[:, 0:1]
            )
            rinv2 = tp.tile([P, 1], dt32, name="rinv2", tag="rinv2")
            nc.vector.reciprocal(rinv2[:, :], stdv[:, :])
            ot = otp.tile([P, H], dt32, name="ot", tag="ot")
            nc.vector.scalar_tensor_tensor(
                ot[:, :], f2[:, :], rinv2[:, 0:1], gbc[:, :],
                op0=ALU.mult, op1=ALU.mult,
            )
            nc.gpsimd.tensor_add(ot[:, :], ot[:, :], bbc[:, :])
            nc.sync.dma_start(out=t["out"][tau * P : (tau + 1) * P, :], in_=ot[:, :])

        # ---------- main expert loop ----------
        for e in range(E):
            if e == E - 3:
                for tau in range(5):
                    emit_xrf(tau)
            w1t, w2t, wft = w_cache.pop(e) if e in w_cache else fetch_weights(e)
            h1t = hp.tile([P, F1T, TPC], dt16, name=f"h1t{e}", tag="h1t")
            eot = ep.tile([P, HT, TPC], dt16, name=f"eot{e}", tag="eot")

            for ch in range(NCH):
                c0, c1 = ch * TCH, (ch + 1) * TCH
                # mm1: h1 = gelu(x @ W1 + b1), feature-major
                for m in range(F1T):
                    ps = ps1p.tile([P, TCH], dt32, name="ps1", tag="ps1")
                    for i in range(HT):
                        nc.tensor.matmul(
                            ps[:, :], w1t[:, i, m * P : (m + 1) * P], xs[i][:, c0:c1],
                            start=(i == 0), stop=(i == HT - 1),
                        )
                    nc.scalar.activation(
                        h1t[:, m, c0:c1], ps[:, :], AF.Gelu,
                        bias=b1_sb[:, e * F1T + m : e * F1T + m + 1],
                    )
                # mm2: eo = (h1 @ W2 + b2) * gate, feature-major
                for i2 in range(HT):
                    ps = ps2p.tile([P, TCH], dt32, name="ps2", tag="ps2")
                    for m in range(F1T):
                        nc.tensor.matmul(
                            ps[:, :], w2t[:, m, i2 * P : (i2 + 1) * P], h1t[:, m, c0:c1],
                            start=(m == 0), stop=(m == F1T - 1),
                        )
                    for bh in range(2):
                        b = ch * 2 + bh
                        j = b * E + e
                        nc.scalar.activation(
                            eot[:, i2, b * SC : (b + 1) * SC],
                            ps[:, bh * SC : (bh + 1) * SC],
                            AF.Identity,
                            bias=e2bs[:, i2 * (B * E) + j : i2 * (B * E) + j + 1],
                            scale=gate_bc[:, j : j + 1],
                        )
                # mm3: fused += eo @ Wf, token-major out
                for tt_ in range(TCH // P):
                    tau = ch * (TCH // P) + tt_
                    for n in range(2):
                        ps = ps3p.tile([P, TCH], dt32, name="ps3", tag="ps3")
                        for i in range(HT):
                            nc.tensor.matmul(
                                ps[:, :],
                                eot[:, i, tau * P : (tau + 1) * P],
                                wft[:, i, n * TCH : (n + 1) * TCH],
                                start=(i == 0), stop=(i == HT - 1),
                            )
                        dst = fused[tau][:, n * TCH : (n + 1) * TCH]
                        if e == 0:
                            nc.vector.tensor_copy(dst, ps[:, :])
                        else:
                            nc.vector.tensor_add(dst, dst, ps[:, :])
                        if e == E - 1 and n == 1:
                            if tau + 3 >= 5 and tau + 3 < TT:
                                emit_xrf(tau + 3)
                            emit_tail(tau)
        sqp.release()
        otp.release()
        tp.release()
        ps3p.release()
        txf.release()


def _prep_inputs(inputs):
    """Host-side sharding/layout prep. Returns per-core input maps."""
    f32 = np.float32

    def bf(x):
        return np.ascontiguousarray(np.asarray(x, dtype=f32)).astype(BF16)

    hs = np.ascontiguousarray(np.asarray(inputs["hidden_states"], dtype=f32))  # [B,S,H]
    Xb = bf(hs.reshape(TOK, H))                                    # [8192, 1024] bf16

    e1_w = np.asarray(inputs["e1_w"], f32)
    e1_b = np.asarray(inputs["e1_b"], f32)
    e2_w = np.asarray(inputs["e2_w"], f32)
    e2_b = np.asarray(inputs["e2_b"], f32)
    f_w = np.asarray(inputs["f_w"], f32)

    common = {
        "xpool": Xb,
        "w1": bf(e1_w).reshape(E, HT, P, F1),
        "w2": bf(e2_w).reshape(E, F1T, P, H),
        "wf": bf(f_w).reshape(E, HT, P, H),
        "b1d": np.ascontiguousarray(e1_b.reshape(E, F1T, P).transpose(2, 0, 1)).reshape(P, E * F1T),
        "e2bTd": np.ascontiguousarray(e2_b.reshape(E, HT, P).transpose(2, 1, 0)).reshape(P, HT * E),
        "a1": bf(inputs["a1_w"]).reshape(HT, P, F1),
        "a2": bf(inputs["a2_w"]).reshape(4, P, 256),
        "a3": bf(inputs["a3_w"]).reshape(2, P, 128),
        "s1": bf(inputs["s1_w"]),
        "s2": bf(inputs["s2_w"]),
        "s3": bf(inputs["s3_w"]),
        "ab1": np.ascontiguousarray(np.asarray(inputs["a1_b"], f32).reshape(4, P).T),
        "ab2": np.ascontiguousarray(np.asarray(inputs["a2_b"], f32).reshape(2, P).T),
        "ab3": np.ascontiguousarray(np.asarray(inputs["a3_b"], f32).reshape(1, P).T),
        "sb1": np.ascontiguousarray(np.asarray(inputs["s1_b"], f32).reshape(64, 1)),
        "sb2": np.ascontiguousarray(np.asarray(inputs["s2_b"], f32).reshape(32, 1)),
        "sb3": np.ascontiguousarray(np.broadcast_to(np.asarray(inputs["s3_b"], f32), (B, E))),
        "ident4": np.eye(4, dtype=f32),
        "fbbc_d": np.ascontiguousarray(np.broadcast_to(np.asarray(inputs["f_b"], f32), (P, H))),
        "gbc_d": np.ascontiguousarray(np.broadcast_to(np.asarray(inputs["ln_g"], f32), (P, H))),
        "bbc_d": np.ascontiguousarray(np.broadcast_to(np.asarray(inputs["ln_b"], f32), (P, H))),
    }

    hsb = Xb.reshape(B, S, H)
    in_maps = []
    for c in range(NCORES):
        shard16 = hsb[:, c * SC : (c + 1) * SC, :]                  # [B, SC, H] bf16
        xsh_c = np.ascontiguousarray(shard16.transpose(2, 0, 1)).reshape(H, TPC)
        xres_c = np.ascontiguousarray(hs[:, c * SC : (c + 1) * SC, :]).reshape(TPC, H)
        m = dict(common)
        m["xsh"] = xsh_c
        m["xres"] = xres_c
        in_maps.append(m)
    return in_maps


def kernel(**inputs) -> np.ndarray:
    nc = _build()
    in_maps = _prep_inputs(inputs)
    res = bass_utils.run_bass_kernel_spmd(nc, in_maps, core_ids=list(range(NCORES)))
    out_full = np.empty((B, S, H), dtype=np.float32)
    for c in range(NCORES):
        out_full[:, c * SC : (c + 1) * SC, :] = res.results[c]["out"].reshape(B, SC, H)
    return out_full


# revision 28
# speedup vs baseline: 1.3615x; 1.3615x over previous
"""Trainium2 Bass kernel for nn_DynamicAdapter (dense-MoE adapter block).

Math (per reference):
  pooled = mean_s(hidden)                               [B, H]
  gate = softmax(MLP_sel(MLP_ana(pooled)))              [B, E]
  h1_e = gelu(x @ W1_e + b1_e)                          [T, H/2]
  eo_e = (h1_e @ W2_e + b2_e) * gate[b, e]              [T, H]
  fused = sum_e eo_e @ Wf_e + f_b + x                   [T, H]
  out = layernorm(fused) * ln_g + ln_b

Sharding: token-parallel. Core c handles tokens {(b, c*256+j)} — 1024 tokens.
Every core runs all 16 experts on its tokens (weights replicated), computes
the gate redundantly from a replicated bf16 copy of x, and writes its token
shard of the output. No collectives.

Layout: activations feature-major ([feat_part, token_free]) through mm1/mm2;
mm3 uses eo tiles as the stationary operand so the fused output comes out
token-major, which makes the residual+LayerNorm tail and output DMA natural.
All matmul operands bf16 (PSUM accumulation fp32); everything else fp32.
"""

import numpy as np
import ml_dtypes

import concourse.bacc as bacc
import concourse.mybir as mybir
import concourse.tile as tile
from concourse import bass_utils

BF16 = ml_dtypes.bfloat16

B, S, H, E = 4, 2048, 1024, 16
NCORES = 8
P = 128
TOK = B * S            # 8192 tokens total
TPC = TOK // NCORES    # 1024 tokens per core
SC = S // NCORES       # 256 tokens per (batch, core)
HT = H // P            # 8 h-tiles
F1 = H // 2            # 512 expert hidden
F1T = F1 // P          # 4 f1-tiles
TCH = 512              # token chunk for mm1/mm2 (2 batch-chunks)
NCH = TPC // TCH       # 2 chunks
TT = TPC // P          # 8 token-tiles for mm3/tail

dt16 = mybir.dt.bfloat16
dt32 = mybir.dt.float32
AF = mybir.ActivationFunctionType
ALU = mybir.AluOpType
AX = mybir.AxisListType

_BUILT = {}


def _build(reps=1):
    if reps in _BUILT:
        return _BUILT[reps]

    nc = bacc.Bacc("TRN2", target_bir_lowering=False, debug=False)

    # ---- kernel I/O ----
    xpool = nc.dram_tensor("xpool", [TOK, H], dt16, kind="ExternalInput").ap()
    xsh = nc.dram_tensor("xsh", [H, TPC], dt16, kind="ExternalInput").ap()
    xres = nc.dram_tensor("xres", [TPC, H], dt32, kind="ExternalInput").ap()
    w1 = nc.dram_tensor("w1", [E, HT, P, F1], dt16, kind="ExternalInput").ap()
    w2 = nc.dram_tensor("w2", [E, F1T, P, H], dt16, kind="ExternalInput").ap()
    wf = nc.dram_tensor("wf", [E, HT, P, H], dt16, kind="ExternalInput").ap()
    b1d = nc.dram_tensor("b1d", [P, E * F1T], dt32, kind="ExternalInput").ap()
    e2bTd = nc.dram_tensor("e2bTd", [P, HT * E], dt32, kind="ExternalInput").ap()
    a1 = nc.dram_tensor("a1", [HT, P, F1], dt16, kind="ExternalInput").ap()
    a2 = nc.dram_tensor("a2", [4, P, 256], dt16, kind="ExternalInput").ap()
    a3 = nc.dram_tensor("a3", [2, P, 128], dt16, kind="ExternalInput").ap()
    s1 = nc.dram_tensor("s1", [P, 64], dt16, kind="ExternalInput").ap()
    s2 = nc.dram_tensor("s2", [64, 32], dt16, kind="ExternalInput").ap()
    s3 = nc.dram_tensor("s3", [32, 16], dt16, kind="ExternalInput").ap()
    ab1 = nc.dram_tensor("ab1", [P, 4], dt32, kind="ExternalInput").ap()
    ab2 = nc.dram_tensor("ab2", [P, 2], dt32, kind="ExternalInput").ap()
    ab3 = nc.dram_tensor("ab3", [P, 1], dt32, kind="ExternalInput").ap()
    sb1 = nc.dram_tensor("sb1", [64, 1], dt32, kind="ExternalInput").ap()
    sb2 = nc.dram_tensor("sb2", [32, 1], dt32, kind="ExternalInput").ap()
    sb3 = nc.dram_tensor("sb3", [B, E], dt32, kind="ExternalInput").ap()
    ident4 = nc.dram_tensor("ident4", [4, 4], dt32, kind="ExternalInput").ap()
    fbbc_d = nc.dram_tensor("fbbc_d", [P, H], dt32, kind="ExternalInput").ap()
    gbc_d = nc.dram_tensor("gbc_d", [P, H], dt32, kind="ExternalInput").ap()
    bbc_d = nc.dram_tensor("bbc_d", [P, H], dt32, kind="ExternalInput").ap()
    out = nc.dram_tensor("out", [TPC, H], dt32, kind="ExternalOutput").ap()

    env = locals()
    with tile.TileContext(nc) as tc:
        for _ in range(reps):
            _emit(tc, env)
    nc.compile()
    _BUILT[reps] = nc
    return nc


def _emit(tc, t):
    nc = tc.nc
    with (
        tc.tile_pool(name="persist", bufs=1) as pp,
        tc.tile_pool(name="wpool", bufs=2) as wp,
        tc.tile_pool(name="hpool", bufs=2) as hp,
        tc.tile_pool(name="eopool", bufs=1) as ep,
        tc.tile_pool(name="ps1p", bufs=2, space="PSUM") as ps1p,
        tc.tile_pool(name="ps2p", bufs=2, space="PSUM") as ps2p,
    ):
        # ---------- critical-path DMAs first: x shard + expert-0 weights ----------
        xs = []
        for i in range(HT):
            xt = pp.tile([P, TPC], dt16, name=f"xs{i}", tag=f"xs{i}")
            nc.sync.dma_start(out=xt[:, :], in_=t["xsh"][i * P : (i + 1) * P, :])
            xs.append(xt)

        def fetch_weights(e):
            w1t = wp.tile([P, HT, F1], dt16, name=f"w1t{e}", tag="w1t")
            src1 = t["w1"][e].rearrange("i p f -> p i f")
            nc.sync.dma_start(out=w1t[:, 0:4, :], in_=src1[:, 0:4, :])
            nc.sync.dma_start(out=w1t[:, 4:8, :], in_=src1[:, 4:8, :])
            w2t = wp.tile([P, F1T, H], dt16, name=f"w2t{e}", tag="w2t")
            src2 = t["w2"][e].rearrange("m p h -> p m h")
            nc.sync.dma_start(out=w2t[:, 0:2, :], in_=src2[:, 0:2, :])
            nc.sync.dma_start(out=w2t[:, 2:4, :], in_=src2[:, 2:4, :])
            wft = wp.tile([P, HT, H], dt16, name=f"wft{e}", tag="wft")
            srcf = t["wf"][e].rearrange("i p g -> p i g")
            nc.sync.dma_start(out=wft[:, 0:2, :], in_=srcf[:, 0:2, :])
            nc.sync.dma_start(out=wft[:, 2:4, :], in_=srcf[:, 2:4, :])
            nc.sync.dma_start(out=wft[:, 4:6, :], in_=srcf[:, 4:6, :])
            nc.sync.dma_start(out=wft[:, 6:8, :], in_=srcf[:, 6:8, :])
            return w1t, w2t, wft

        w_cache = {0: fetch_weights(0)}
        b1_sb = pp.tile([P, E * F1T], dt32, name="b1_sb", tag="b1_sb")
        nc.sync.dma_start(out=b1_sb[:, :], in_=t["b1d"][:, :])
        e2bT_sb = pp.tile([P, HT * E], dt32, name="e2bT_sb", tag="e2bT_sb")
        nc.sync.dma_start(out=e2bT_sb[:, :], in_=t["e2bTd"][:, :])
        id4 = pp.tile([4, 4], dt32, name="id4", tag="id4")
        nc.sync.dma_start(out=id4[:, :], in_=t["ident4"][:, :])
        eps = pp.tile([P, 1], dt32, name="eps", tag="eps")
        nc.vector.memset(eps[:, :], 1e-5)

        fused = []
        for tau in range(TT):
            ft = pp.tile([P, H], dt32, name=f"fused{tau}", tag=f"fused{tau}")
            fused.append(ft)

        # per-(b,e) gate broadcast to all partitions, and gate-scaled e2 bias
        gate_bc = pp.tile([P, B * E], dt32, name="gate_bc", tag="gate_bc")
        e2bs = pp.tile([P, HT * B * E], dt32, name="e2bs", tag="e2bs")

        # batch-indicator stationaries for pooling: column b holds 1/S
        ind = pp.tile([P, B], dt16, name="ind", tag="ind")
        nc.vector.memset(ind[:, :], 0.0)
        for b in range(B):
            nc.vector.memset(ind[:, b : b + 1], 1.0 / S)

        # ---------- phase 0: pooling + gate ----------
        with (
            tc.tile_pool(name="ph0", bufs=6) as xp,
            tc.tile_pool(name="gw", bufs=1) as gw,
            tc.tile_pool(name="ps0p", bufs=2, space="PSUM") as ps0p,
            tc.tile_pool(name="psgp", bufs=2, space="PSUM") as psgp,
        ):
            # gate weights first: small, and needed as soon as pooling lands
            a1_sb = gw.tile([P, HT, F1], dt16, name="a1_sb", tag="a1_sb")
            nc.sync.dma_start(out=a1_sb[:, :, :], in_=t["a1"].rearrange("i p f -> p i f"))
            a2_sb = gw.tile([P, 4, 256], dt16, name="a2_sb", tag="a2_sb")
            nc.sync.dma_start(out=a2_sb[:, :, :], in_=t["a2"].rearrange("i p f -> p i f"))
            a3_sb = gw.tile([P, 2, 128], dt16, name="a3_sb", tag="a3_sb")
            nc.sync.dma_start(out=a3_sb[:, :, :], in_=t["a3"].rearrange("i p f -> p i f"))
            s1_sb = gw.tile([P, 64], dt16, name="s1_sb", tag="s1_sb")
            nc.sync.dma_start(out=s1_sb[:, :], in_=t["s1"][:, :])
            s2_sb = gw.tile([64, 32], dt16, name="s2_sb", tag="s2_sb")
            nc.sync.dma_start(out=s2_sb[:, :], in_=t["s2"][:, :])
            s3_sb = gw.tile([32, 16], dt16, name="s3_sb", tag="s3_sb")
            nc.sync.dma_start(out=s3_sb[:, :], in_=t["s3"][:, :])
            ab1_sb = gw.tile([P, 4], dt32, name="ab1_sb", tag="ab1_sb")
            nc.sync.dma_start(out=ab1_sb[:, :], in_=t["ab1"][:, :])
            ab2_sb = gw.tile([P, 2], dt32, name="ab2_sb", tag="ab2_sb")
            nc.sync.dma_start(out=ab2_sb[:, :], in_=t["ab2"][:, :])
            ab3_sb = gw.tile([P, 1], dt32, name="ab3_sb", tag="ab3_sb")
            nc.sync.dma_start(out=ab3_sb[:, :], in_=t["ab3"][:, :])
            sb1_sb = gw.tile([64, 1], dt32, name="sb1_sb", tag="sb1_sb")
            nc.sync.dma_start(out=sb1_sb[:, :], in_=t["sb1"][:, :])
            sb2_sb = gw.tile([32, 1], dt32, name="sb2_sb", tag="sb2_sb")
            nc.sync.dma_start(out=sb2_sb[:, :], in_=t["sb2"][:, :])
            sb3_sb = gw.tile([B, E], dt32, name="sb3_sb", tag="sb3_sb")
            nc.sync.dma_start(out=sb3_sb[:, :], in_=t["sb3"][:, :])

            # mean-pool over the full sequence: indicator-column matmuls
            # accumulate all 64 token-tiles into one [B, 512] psum per h-half.
            stack = gw.tile([B, H], dt32, name="stack", tag="stack")
            for hc in range(2):
                psp = ps0p.tile([B, TCH], dt32, name="psp", tag="ps0")
                n_t = TOK // P
                for tt_ in range(n_t):
                    b = tt_ // (S // P)
                    xt = xp.tile([P, TCH], dt16, name="xt", tag="xt")
                    nc.sync.dma_start(
                        out=xt[:, :],
                        in_=t["xpool"][tt_ * P : (tt_ + 1) * P, hc * TCH : (hc + 1) * TCH],
                    )
                    nc.tensor.matmul(
                        psp[:, :], ind[:, :], xt[:, :],
                        start=(tt_ == 0), stop=(tt_ == n_t - 1),
                    )
                nc.scalar.copy(stack[:, hc * TCH : (hc + 1) * TCH], psp[:, :])

            # transpose pooled [B, H] -> 8 feature-major [P, B] bf16 tiles
            ptb = []
            for i in range(HT):
                pst = psgp.tile([P, B], dt32, name="pst", tag="psg")
                nc.tensor.transpose(pst[:, :], stack[:, i * P : (i + 1) * P], id4[:, :])
                pb = gw.tile([P, B], dt16, name=f"ptb{i}", tag=f"ptb{i}")
                nc.scalar.copy(pb[:, :], pst[:, :])
                ptb.append(pb)

            # gate MLP (feature-major)
            t1 = gw.tile([P, 16], dt16, name="t1", tag="t1")
            for m in range(4):
                psg = psgp.tile([P, B], dt32, name="psg1", tag="psg")
                for i in range(HT):
                    nc.tensor.matmul(
                        psg[:, :], a1_sb[:, i, m * P : (m + 1) * P], ptb[i][:, :],
                        start=(i == 0), stop=(i == HT - 1),
                    )
                nc.scalar.activation(
                    t1[:, m * B : (m + 1) * B], psg[:, :], AF.Gelu,
                    bias=ab1_sb[:, m : m + 1],
                )
            t2 = gw.tile([P, 8], dt16, name="t2", tag="t2")
            for m in range(2):
                psg = psgp.tile([P, B], dt32, name="psg2", tag="psg")
                for i in range(4):
                    nc.tensor.matmul(
                        psg[:, :], a2_sb[:, i, m * P : (m + 1) * P], t1[:, i * B : (i + 1) * B],
                        start=(i == 0), stop=(i == 3),
                    )
                nc.scalar.activation(
                    t2[:, m * B : (m + 1) * B], psg[:, :], AF.Gelu,
                    bias=ab2_sb[:, m : m + 1],
                )
            t3 = gw.tile([P, B], dt16, name="t3", tag="t3")
            psg = psgp.tile([P, B], dt32, name="psg3", tag="psg")
            for i in range(2):
                nc.tensor.matmul(
                    psg[:, :], a3_sb[:, i, :], t2[:, i * B : (i + 1) * B],
                    start=(i == 0), stop=(i == 1),
                )
            nc.scalar.activation(t3[:, :], psg[:, :], AF.Identity, bias=ab3_sb[:, 0:1])

            g1 = gw.tile([64, B], dt16, name="g1", tag="g1")
            psg = psgp.tile([64, B], dt32, name="psg4", tag="psg")
            nc.tensor.matmul(psg[:, :], s1_sb[:, :], t3[:, :], start=True, stop=True)
            nc.scalar.activation(g1[:, :], psg[:, :], AF.Gelu, bias=sb1_sb[:, 0:1])

            g2 = gw.tile([32, B], dt16, name="g2", tag="g2")
            psg = psgp.tile([32, B], dt32, name="psg5", tag="psg")
            nc.tensor.matmul(psg[:, :], s2_sb[:, :], g1[:, :], start=True, stop=True)
            nc.scalar.activation(g2[:, :], psg[:, :], AF.Gelu, bias=sb2_sb[:, 0:1])

            # flip to token-major: z[b, e]
            z = gw.tile([B, E], dt32, name="z", tag="z")
            psg = psgp.tile([B, E], dt32, name="psg6", tag="psg")
            nc.tensor.matmul(psg[:, :], g2[:, :], s3_sb[:, :], start=True, stop=True)
            nc.scalar.copy(z[:, :], psg[:, :])
            nc.vector.tensor_add(z[:, :], z[:, :], sb3_sb[:, :])

            # softmax over E (free dim)
            mx = gw.tile([B, 1], dt32, name="mx", tag="mx")
            nc.vector.reduce_max(mx[:, :], z[:, :], axis=AX.X)
            nc.vector.tensor_scalar_sub(z[:, :], z[:, :], mx[:, 0:1])
            sums = gw.tile([B, 1], dt32, name="sums", tag="sums")
            exps = gw.tile([B, E], dt32, name="exps", tag="exps")
            nc.scalar.activation(exps[:, :], z[:, :], AF.Exp, accum_out=sums[:, 0:1])
            rinv = gw.tile([B, 1], dt32, name="rinv", tag="rinv")
            nc.vector.reciprocal(rinv[:, :], sums[:, :])
            gate4 = gw.tile([B, E], dt32, name="gate4", tag="gate4")
            nc.vector.tensor_scalar_mul(gate4[:, :], exps[:, :], rinv[:, 0:1])

            # broadcast gate to all 128 partitions via DRAM bounce
            with tc.tile_pool(name="dramp", bufs=1, space="DRAM") as dp:
                gsc = dp.tile([1, B * E], dt32, name="gsc", tag="gsc")
                nc.sync.dma_start(
                    out=gsc.rearrange("o (b e) -> (o b) e", b=B), in_=gate4[:, :]
                )
                gflat = gw.tile([1, B * E], dt32, name="gflat", tag="gflat")
                nc.sync.dma_start(out=gflat[:, :], in_=gsc[:, :])
            nc.gpsimd.partition_broadcast(gate_bc[:, :], gflat[:, :])

            # gate-scaled e2 bias: e2bs[p, (i, b, e)] = e2bT[p, (i, e)] * gate[b, e]
            for i in range(HT):
                for b in range(B):
                    nc.vector.tensor_mul(
                        e2bs[:, i * (B * E) + b * E : i * (B * E) + (b + 1) * E],
                        e2bT_sb[:, i * E : (i + 1) * E],
                        gate_bc[:, b * E : (b + 1) * E],
                    )

        # ---------- tail constants (needed only at the end) ----------
        fbbc = pp.tile([P, H], dt32, name="fbbc", tag="fbbc")
        nc.sync.dma_start(out=fbbc[:, :], in_=t["fbbc_d"][:, :])
        gbc = pp.tile([P, H], dt32, name="gbc", tag="gbc")
        nc.sync.dma_start(out=gbc[:, :], in_=t["gbc_d"][:, :])
        bbc = pp.tile([P, H], dt32, name="bbc", tag="bbc")
        nc.sync.dma_start(out=bbc[:, :], in_=t["bbc_d"][:, :])

        # ---------- pools for the expert loop + interleaved tail ----------
        txf = tc.alloc_tile_pool(name="txf", bufs=5)
        ps3p = tc.alloc_tile_pool(name="ps3p", bufs=4, space="PSUM")
        tp = tc.alloc_tile_pool(name="tail", bufs=2)
        otp = tc.alloc_tile_pool(name="otp", bufs=2)
        sqp = tc.alloc_tile_pool(name="sqp", bufs=1)
        xrfs = {}

        def emit_xrf(tau):
            # residual + f_b prep; no expert-loop deps, runs on DVE slack
            xrf = txf.tile([P, H], dt32, name=f"xrf{tau}", tag="xrf")
            nc.sync.dma_start(out=xrf[:, :], in_=t["xres"][tau * P : (tau + 1) * P, :])
            nc.vector.tensor_add(xrf[:, :], xrf[:, :], fbbc[:, :])
            xrfs[tau] = xrf

        def emit_tail(tau):
            # layernorm tail for one token tile; DVE: +xrf, reduce, scale*g;
            # ACT: center, square-accum, sqrt; GpSimd: +ln_b.
            f2 = fused[tau]
            nc.vector.tensor_add(f2[:, :], f2[:, :], xrfs[tau][:, :])
            ssum = tp.tile([P, 1], dt32, name="ssum", tag="ssum")
            nc.vector.reduce_sum(ssum[:, :], f2[:, :], axis=AX.X)
            negmu = tp.tile([P, 1], dt32, name="negmu", tag="negmu")
            nc.vector.tensor_scalar_mul(negmu[:, :], ssum[:, :], -1.0 / H)
            nc.scalar.activation(f2[:, :], f2[:, :], AF.Identity, bias=negmu[:, 0:1])
            sq = sqp.tile([P, H], dt16, name="sq", tag="sq")
            ssq = tp.tile([P, 1], dt32, name="ssq", tag="ssq")
            nc.scalar.activation(sq[:, :], f2[:, :], AF.Square, accum_out=ssq[:, 0:1])
            stdv = tp.tile([P, 1], dt32, name="stdv", tag="stdv")
            nc.scalar.activation(
                stdv[:, :], ssq[:, :], AF.Sqrt, scale=1.0 / H, bias=eps[:, 0:1]
            )
            rinv2 = tp.tile([P, 1], dt32, name="rinv2", tag="rinv2")
            nc.vector.reciprocal(rinv2[:, :], stdv[:, :])
            ot = otp.tile([P, H], dt32, name="ot", tag="ot")
            nc.vector.scalar_tensor_tensor(
                ot[:, :], f2[:, :], rinv2[:, 0:1], gbc[:, :],
                op0=ALU.mult, op1=ALU.mult,
            )
            nc.gpsimd.tensor_add(ot[:, :], ot[:, :], bbc[:, :])
            nc.sync.dma_start(out=t["out"][tau * P : (tau + 1) * P, :], in_=ot[:, :])

        # ---------- main expert loop ----------
        for e in range(E):
            if e == E - 3:
                for tau in range(5):
                    emit_xrf(tau)
            w1t, w2t, wft = w_cache.pop(e) if e in w_cache else fetch_weights(e)
            h1t = hp.tile([P, F1T, TPC], dt16, name=f"h1t{e}", tag="h1t")
            eot = ep.tile([P, HT, TPC], dt16, name=f"eot{e}", tag="eot")

            for ch in range(NCH):
                c0, c1 = ch * TCH, (ch + 1) * TCH
                # mm1: h1 = gelu(x @ W1 + b1), feature-major
                for m in range(F1T):
                    ps = ps1p.tile([P, TCH], dt32, name="ps1", tag="ps1")
                    for i in range(HT):
                        nc.tensor.matmul(
                            ps[:, :], w1t[:, i, m * P : (m + 1) * P], xs[i][:, c0:c1],
                            start=(i == 0), stop=(i == HT - 1),
                        )
                    nc.scalar.activation(
                        h1t[:, m, c0:c1], ps[:, :], AF.Gelu,
                        bias=b1_sb[:, e * F1T + m : e * F1T + m + 1],
                    )
                # mm2: eo = (h1 @ W2 + b2) * gate, feature-major
                for i2 in range(HT):
                    ps = ps2p.tile([P, TCH], dt32, name="ps2", tag="ps2")
                    for m in range(F1T):
                        nc.tensor.matmul(
                            ps[:, :], w2t[:, m, i2 * P : (i2 + 1) * P], h1t[:, m, c0:c1],
                            start=(m == 0), stop=(m == F1T - 1),
                        )
                    for bh in range(2):
                        b = ch * 2 + bh
                        j = b * E + e
                        nc.scalar.activation(
                            eot[:, i2, b * SC : (b + 1) * SC],
                            ps[:, bh * SC : (bh + 1) * SC],
                            AF.Identity,
                            bias=e2bs[:, i2 * (B * E) + j : i2 * (B * E) + j + 1],
                            scale=gate_bc[:, j : j + 1],
                        )
                # mm3: fused += eo @ Wf, token-major out
                for tt_ in range(TCH // P):
                    tau = ch * (TCH // P) + tt_
                    for n in range(2):
                        ps = ps3p.tile([P, TCH], dt32, name="ps3", tag="ps3")
                        for i in range(HT):
                            nc.tensor.matmul(
                                ps[:, :],
                                eot[:, i, tau * P : (tau + 1) * P],
                                wft[:, i, n * TCH : (n + 1) * TCH],
                                start=(i == 0), stop=(i == HT - 1),
                            )
                        dst = fused[tau][:, n * TCH : (n + 1) * TCH]
                        if e == 0:
                            nc.vector.tensor_copy(dst, ps[:, :])
                        else:
                            nc.vector.tensor_add(dst, dst, ps[:, :])
                        if e == E - 1 and n == 1:
                            if tau + 3 >= 5 and tau + 3 < TT:
                                emit_xrf(tau + 3)
                            emit_tail(tau)
        sqp.release()
        otp.release()
        tp.release()
        ps3p.release()
        txf.release()


def _prep_inputs(inputs):
    """Host-side sharding/layout prep. Returns per-core input maps."""
    f32 = np.float32

    def bf(x):
        return np.ascontiguousarray(np.asarray(x, dtype=f32)).astype(BF16)

    hs = np.ascontiguousarray(np.asarray(inputs["hidden_states"], dtype=f32))  # [B,S,H]
    Xb = bf(hs.reshape(TOK, H))                                    # [8192, 1024] bf16

    e1_w = np.asarray(inputs["e1_w"], f32)
    e1_b = np.asarray(inputs["e1_b"], f32)
    e2_w = np.asarray(inputs["e2_w"], f32)
    e2_b = np.asarray(inputs["e2_b"], f32)
    f_w = np.asarray(inputs["f_w"], f32)

    common = {
        "xpool": Xb,
        "w1": bf(e1_w).reshape(E, HT, P, F1),
        "w2": bf(e2_w).reshape(E, F1T, P, H),
        "wf": bf(f_w).reshape(E, HT, P, H),
        "b1d": np.ascontiguousarray(e1_b.reshape(E, F1T, P).transpose(2, 0, 1)).reshape(P, E * F1T),
        "e2bTd": np.ascontiguousarray(e2_b.reshape(E, HT, P).transpose(2, 1, 0)).reshape(P, HT * E),
        "a1": bf(inputs["a1_w"]).reshape(HT, P, F1),
        "a2": bf(inputs["a2_w"]).reshape(4, P, 256),
        "a3": bf(inputs["a3_w"]).reshape(2, P, 128),
        "s1": bf(inputs["s1_w"]),
        "s2": bf(inputs["s2_w"]),
        "s3": bf(inputs["s3_w"]),
        "ab1": np.ascontiguousarray(np.asarray(inputs["a1_b"], f32).reshape(4, P).T),
        "ab2": np.ascontiguousarray(np.asarray(inputs["a2_b"], f32).reshape(2, P).T),
        "ab3": np.ascontiguousarray(np.asarray(inputs["a3_b"], f32).reshape(1, P).T),
        "sb1": np.ascontiguousarray(np.asarray(inputs["s1_b"], f32).reshape(64, 1)),
        "sb2": np.ascontiguousarray(np.asarray(inputs["s2_b"], f32).reshape(32, 1)),
        "sb3": np.ascontiguousarray(np.broadcast_to(np.asarray(inputs["s3_b"], f32), (B, E))),
        "ident4": np.eye(4, dtype=f32),
        "fbbc_d": np.ascontiguousarray(np.broadcast_to(np.asarray(inputs["f_b"], f32), (P, H))),
        "gbc_d": np.ascontiguousarray(np.broadcast_to(np.asarray(inputs["ln_g"], f32), (P, H))),
        "bbc_d": np.ascontiguousarray(np.broadcast_to(np.asarray(inputs["ln_b"], f32), (P, H))),
    }

    hsb = Xb.reshape(B, S, H)
    in_maps = []
    for c in range(NCORES):
        shard16 = hsb[:, c * SC : (c + 1) * SC, :]                  # [B, SC, H] bf16
        xsh_c = np.ascontiguousarray(shard16.transpose(2, 0, 1)).reshape(H, TPC)
        xres_c = np.ascontiguousarray(hs[:, c * SC : (c + 1) * SC, :]).reshape(TPC, H)
        m = dict(common)
        m["xsh"] = xsh_c
        m["xres"] = xres_c
        in_maps.append(m)
    return in_maps


def kernel(**inputs) -> np.ndarray:
    nc = _build()
    in_maps = _prep_inputs(inputs)
    res = bass_utils.run_bass_kernel_spmd(nc, in_maps, core_ids=list(range(NCORES)))
    out_full = np.empty((B, S, H), dtype=np.float32)
    for c in range(NCORES):
        out_full[:, c * SC : (c + 1) * SC, :] = res.results[c]["out"].reshape(B, SC, H)
    return out_full


# revision 37
# speedup vs baseline: 1.7024x; 1.2504x over previous
"""Trainium2 Bass kernel for nn_DynamicAdapter (dense-MoE adapter block).

Math (per reference):
  pooled = mean_s(hidden)                               [B, H]
  gate = softmax(MLP_sel(MLP_ana(pooled)))              [B, E]
  h1_e = gelu(x @ W1_e + b1_e)                          [T, H/2]
  eo_e = (h1_e @ W2_e + b2_e) * gate[b, e]              [T, H]
  fused = sum_e eo_e @ Wf_e + f_b + x                   [T, H]
  out = layernorm(fused) * ln_g + ln_b

Sharding: token-parallel. Core c handles tokens {(b, c*256+j)} — 1024 tokens.
Every core runs all 16 experts on its tokens (weights replicated), computes
the gate redundantly from a replicated bf16 copy of x, and writes its token
shard of the output. No collectives.

Layout: activations feature-major ([feat_part, token_free]) through mm1/mm2;
mm3 uses eo tiles as the stationary operand so the fused output comes out
token-major, which makes the residual+LayerNorm tail and output DMA natural.
All matmul operands bf16 (PSUM accumulation fp32); everything else fp32.
"""

import numpy as np
import ml_dtypes

import concourse.bacc as bacc
import concourse.mybir as mybir
import concourse.tile as tile
from concourse import bass_utils

BF16 = ml_dtypes.bfloat16

B, S, H, E = 4, 2048, 1024, 16
NCORES = 8
P = 128
TOK = B * S            # 8192 tokens total
TPC = TOK // NCORES    # 1024 tokens per core
SC = S // NCORES       # 256 tokens per (batch, core)
HT = H // P            # 8 h-tiles
F1 = H // 2            # 512 expert hidden
F1T = F1 // P          # 4 f1-tiles
TCH = 512              # token chunk for mm1/mm2 (2 batch-chunks)
NCH = TPC // TCH       # 2 chunks
TT = TPC // P          # 8 token-tiles for mm3/tail

dt16 = mybir.dt.bfloat16
dt32 = mybir.dt.float32
AF = mybir.ActivationFunctionType
ALU = mybir.AluOpType
AX = mybir.AxisListType

# Pool via per-core partial sums + a 16KB AllReduce instead of each core
# reading the full replicated activation tensor.
USE_COLLECTIVE = True

_BUILT = {}


def _build(reps=1):
    if reps in _BUILT:
        return _BUILT[reps]

    nc = bacc.Bacc("TRN2", target_bir_lowering=False, debug=False)

    # ---- kernel I/O ----
    if not USE_COLLECTIVE:
        xpool = nc.dram_tensor("xpool", [TOK, H], dt16, kind="ExternalInput").ap()
    xsh = nc.dram_tensor("xsh", [H, TPC], dt16, kind="ExternalInput").ap()
    xres = nc.dram_tensor("xres", [TPC, H], dt32, kind="ExternalInput").ap()
    w1 = nc.dram_tensor("w1", [E, HT, P, F1], dt16, kind="ExternalInput").ap()
    w2 = nc.dram_tensor("w2", [E, F1T, P, H], dt16, kind="ExternalInput").ap()
    wf = nc.dram_tensor("wf", [E, HT, P, H], dt16, kind="ExternalInput").ap()
    b1d = nc.dram_tensor("b1d", [P, E * F1T], dt32, kind="ExternalInput").ap()
    e2bTd = nc.dram_tensor("e2bTd", [P, HT * E], dt32, kind="ExternalInput").ap()
    a1 = nc.dram_tensor("a1", [HT, P, F1], dt16, kind="ExternalInput").ap()
    a2 = nc.dram_tensor("a2", [4, P, 256], dt16, kind="ExternalInput").ap()
    a3 = nc.dram_tensor("a3", [2, P, 128], dt16, kind="ExternalInput").ap()
    s1 = nc.dram_tensor("s1", [P, 64], dt16, kind="ExternalInput").ap()
    s2 = nc.dram_tensor("s2", [64, 32], dt16, kind="ExternalInput").ap()
    s3 = nc.dram_tensor("s3", [32, 16], dt16, kind="ExternalInput").ap()
    ab1 = nc.dram_tensor("ab1", [P, 4], dt32, kind="ExternalInput").ap()
    ab2 = nc.dram_tensor("ab2", [P, 2], dt32, kind="ExternalInput").ap()
    ab3 = nc.dram_tensor("ab3", [P, 1], dt32, kind="ExternalInput").ap()
    sb1 = nc.dram_tensor("sb1", [64, 1], dt32, kind="ExternalInput").ap()
    sb2 = nc.dram_tensor("sb2", [32, 1], dt32, kind="ExternalInput").ap()
    sb3 = nc.dram_tensor("sb3", [B, E], dt32, kind="ExternalInput").ap()
    if not USE_COLLECTIVE:
        ident4 = nc.dram_tensor("ident4", [4, 4], dt32, kind="ExternalInput").ap()
    fbbc_d = nc.dram_tensor("fbbc_d", [P, H], dt32, kind="ExternalInput").ap()
    gbc_d = nc.dram_tensor("gbc_d", [P, H], dt32, kind="ExternalInput").ap()
    bbc_d = nc.dram_tensor("bbc_d", [P, H], dt32, kind="ExternalInput").ap()
    out = nc.dram_tensor("out", [TPC, H], dt32, kind="ExternalOutput").ap()

    env = locals()
    with tile.TileContext(nc) as tc:
        for _ in range(reps):
            _emit(tc, env)
    nc.compile()
    _BUILT[reps] = nc
    return nc


def _emit(tc, t):
    nc = tc.nc
    with (
        tc.tile_pool(name="persist", bufs=1) as pp,
        tc.tile_pool(name="wpool", bufs=2) as wp,
        tc.tile_pool(name="hpool", bufs=2) as hp,
        tc.tile_pool(name="eopool", bufs=1) as ep,
        tc.tile_pool(name="ps1p", bufs=2, space="PSUM") as ps1p,
        tc.tile_pool(name="ps2p", bufs=2, space="PSUM") as ps2p,
    ):
        # ---------- critical-path DMAs first: x shard + expert-0 weights ----------
        xs = []
        for i in range(HT):
            xt = pp.tile([P, TPC], dt16, name=f"xs{i}", tag=f"xs{i}")
            nc.sync.dma_start(out=xt[:, :], in_=t["xsh"][i * P : (i + 1) * P, :])
            xs.append(xt)

        def fetch_weights(e):
            w1t = wp.tile([P, HT, F1], dt16, name=f"w1t{e}", tag="w1t")
            src1 = t["w1"][e].rearrange("i p f -> p i f")
            nc.sync.dma_start(out=w1t[:, 0:4, :], in_=src1[:, 0:4, :])
            nc.sync.dma_start(out=w1t[:, 4:8, :], in_=src1[:, 4:8, :])
            w2t = wp.tile([P, F1T, H], dt16, name=f"w2t{e}", tag="w2t")
            src2 = t["w2"][e].rearrange("m p h -> p m h")
            nc.sync.dma_start(out=w2t[:, 0:2, :], in_=src2[:, 0:2, :])
            nc.sync.dma_start(out=w2t[:, 2:4, :], in_=src2[:, 2:4, :])
            wft = wp.tile([P, HT, H], dt16, name=f"wft{e}", tag="wft")
            srcf = t["wf"][e].rearrange("i p g -> p i g")
            nc.sync.dma_start(out=wft[:, 0:2, :], in_=srcf[:, 0:2, :])
            nc.sync.dma_start(out=wft[:, 2:4, :], in_=srcf[:, 2:4, :])
            nc.sync.dma_start(out=wft[:, 4:6, :], in_=srcf[:, 4:6, :])
            nc.sync.dma_start(out=wft[:, 6:8, :], in_=srcf[:, 6:8, :])
            return w1t, w2t, wft

        w_cache = {0: fetch_weights(0)}
        b1_sb = pp.tile([P, E * F1T], dt32, name="b1_sb", tag="b1_sb")
        nc.sync.dma_start(out=b1_sb[:, :], in_=t["b1d"][:, :])
        e2bT_sb = pp.tile([P, HT * E], dt32, name="e2bT_sb", tag="e2bT_sb")
        nc.sync.dma_start(out=e2bT_sb[:, :], in_=t["e2bTd"][:, :])
        if not USE_COLLECTIVE:
            id4 = pp.tile([4, 4], dt32, name="id4", tag="id4")
            nc.sync.dma_start(out=id4[:, :], in_=t["ident4"][:, :])
        eps = pp.tile([P, 1], dt32, name="eps", tag="eps")
        nc.vector.memset(eps[:, :], 1e-5)

        fused = []
        for tau in range(TT):
            ft = pp.tile([P, H], dt32, name=f"fused{tau}", tag=f"fused{tau}")
            fused.append(ft)

        # per-(b,e) gate broadcast to all partitions, and gate-scaled e2 bias
        gate_bc = pp.tile([P, B * E], dt32, name="gate_bc", tag="gate_bc")
        e2bs = pp.tile([P, HT * B * E], dt32, name="e2bs", tag="e2bs")

        if not USE_COLLECTIVE:
            # batch-indicator stationaries for pooling: column b holds 1/S
            ind = pp.tile([P, B], dt16, name="ind", tag="ind")
            nc.vector.memset(ind[:, :], 0.0)
            for b in range(B):
                nc.vector.memset(ind[:, b : b + 1], 1.0 / S)

        # ---------- phase 0: pooling + gate ----------
        with (
            tc.tile_pool(name="ph0", bufs=6) as xp,
            tc.tile_pool(name="gw", bufs=1) as gw,
            tc.tile_pool(name="ps0p", bufs=2, space="PSUM") as ps0p,
            tc.tile_pool(name="psgp", bufs=2, space="PSUM") as psgp,
        ):
            # gate weights first: small, and needed as soon as pooling lands
            a1_sb = gw.tile([P, HT, F1], dt16, name="a1_sb", tag="a1_sb")
            nc.sync.dma_start(out=a1_sb[:, :, :], in_=t["a1"].rearrange("i p f -> p i f"))
            a2_sb = gw.tile([P, 4, 256], dt16, name="a2_sb", tag="a2_sb")
            nc.sync.dma_start(out=a2_sb[:, :, :], in_=t["a2"].rearrange("i p f -> p i f"))
            a3_sb = gw.tile([P, 2, 128], dt16, name="a3_sb", tag="a3_sb")
            nc.sync.dma_start(out=a3_sb[:, :, :], in_=t["a3"].rearrange("i p f -> p i f"))
            s1_sb = gw.tile([P, 64], dt16, name="s1_sb", tag="s1_sb")
            nc.sync.dma_start(out=s1_sb[:, :], in_=t["s1"][:, :])
            s2_sb = gw.tile([64, 32], dt16, name="s2_sb", tag="s2_sb")
            nc.sync.dma_start(out=s2_sb[:, :], in_=t["s2"][:, :])
            s3_sb = gw.tile([32, 16], dt16, name="s3_sb", tag="s3_sb")
            nc.sync.dma_start(out=s3_sb[:, :], in_=t["s3"][:, :])
            ab1_sb = gw.tile([P, 4], dt32, name="ab1_sb", tag="ab1_sb")
            nc.sync.dma_start(out=ab1_sb[:, :], in_=t["ab1"][:, :])
            ab2_sb = gw.tile([P, 2], dt32, name="ab2_sb", tag="ab2_sb")
            nc.sync.dma_start(out=ab2_sb[:, :], in_=t["ab2"][:, :])
            ab3_sb = gw.tile([P, 1], dt32, name="ab3_sb", tag="ab3_sb")
            nc.sync.dma_start(out=ab3_sb[:, :], in_=t["ab3"][:, :])
            sb1_sb = gw.tile([64, 1], dt32, name="sb1_sb", tag="sb1_sb")
            nc.sync.dma_start(out=sb1_sb[:, :], in_=t["sb1"][:, :])
            sb2_sb = gw.tile([32, 1], dt32, name="sb2_sb", tag="sb2_sb")
            nc.sync.dma_start(out=sb2_sb[:, :], in_=t["sb2"][:, :])
            sb3_sb = gw.tile([B, E], dt32, name="sb3_sb", tag="sb3_sb")
            nc.sync.dma_start(out=sb3_sb[:, :], in_=t["sb3"][:, :])

            if USE_COLLECTIVE:
                # per-core partial pooling over own token shard (already in
                # SBUF), then a 16KB AllReduce across the 8 cores.
                pooled_my = gw.tile([P, HT * B], dt32, name="pooled_my", tag="pooled_my")
                for i in range(HT):
                    for b in range(B):
                        nc.vector.reduce_sum(
                            pooled_my[:, i * B + b : i * B + b + 1],
                            xs[i][:, b * SC : (b + 1) * SC],
                            axis=AX.X,
                        )
                with tc.tile_pool(name="drac", bufs=1, space="DRAM") as dpc:
                    arin = dpc.tile([P, HT * B], dt32, name="arin", tag="arin")
                    arout = dpc.tile(
                        [P, HT * B], dt32, name="arout", tag="arout",
                        addr_space="Shared",
                    )
                    nc.sync.dma_start(out=arin[:, :], in_=pooled_my[:, :])
                    nc.gpsimd.collective_compute(
                        "AllReduce",
                        ALU.add,
                        replica_groups=[list(range(NCORES))],
                        ins=[arin.opt()],
                        outs=[arout.opt()],
                    )
                    pooled_sum = gw.tile(
                        [P, HT * B], dt32, name="pooled_sum", tag="pooled_sum"
                    )
                    nc.sync.dma_start(out=pooled_sum[:, :], in_=arout[:, :])
                ptb = []
                for i in range(HT):
                    pb = gw.tile([P, B], dt16, name=f"ptb{i}", tag=f"ptb{i}")
                    nc.scalar.mul(pb[:, :], pooled_sum[:, i * B : (i + 1) * B], 1.0 / S)
                    ptb.append(pb)
            else:
                # mean-pool over the full sequence: indicator-column matmuls
                # accumulate all 64 token-tiles into one [B, 512] psum per h-half.
                stack = gw.tile([B, H], dt32, name="stack", tag="stack")
                for hc in range(2):
                    psp = ps0p.tile([B, TCH], dt32, name="psp", tag="ps0")
                    n_t = TOK // P
                    for tt_ in range(n_t):
                        b = tt_ // (S // P)
                        xt = xp.tile([P, TCH], dt16, name="xt", tag="xt")
                        nc.sync.dma_start(
                            out=xt[:, :],
                            in_=t["xpool"][tt_ * P : (tt_ + 1) * P, hc * TCH : (hc + 1) * TCH],
                        )
                        nc.tensor.matmul(
                            psp[:, :], ind[:, :], xt[:, :],
                            start=(tt_ == 0), stop=(tt_ == n_t - 1),
                        )
                    nc.scalar.copy(stack[:, hc * TCH : (hc + 1) * TCH], psp[:, :])

                # transpose pooled [B, H] -> 8 feature-major [P, B] bf16 tiles
                ptb = []
                for i in range(HT):
                    pst = psgp.tile([P, B], dt32, name="pst", tag="psg")
                    nc.tensor.transpose(pst[:, :], stack[:, i * P : (i + 1) * P], id4[:, :])
                    pb = gw.tile([P, B], dt16, name=f"ptb{i}", tag=f"ptb{i}")
                    nc.scalar.copy(pb[:, :], pst[:, :])
                    ptb.append(pb)

            # gate MLP (feature-major)
            t1 = gw.tile([P, 16], dt16, name="t1", tag="t1")
            for m in range(4):
                psg = psgp.tile([P, B], dt32, name="psg1", tag="psg")
                for i in range(HT):
                    nc.tensor.matmul(
                        psg[:, :], a1_sb[:, i, m * P : (m + 1) * P], ptb[i][:, :],
                        start=(i == 0), stop=(i == HT - 1),
                    )
                nc.scalar.activation(
                    t1[:, m * B : (m + 1) * B], psg[:, :], AF.Gelu,
                    bias=ab1_sb[:, m : m + 1],
                )
            t2 = gw.tile([P, 8], dt16, name="t2", tag="t2")
            for m in range(2):
                psg = psgp.tile([P, B], dt32, name="psg2", tag="psg")
                for i in range(4):
                    nc.tensor.matmul(
                        psg[:, :], a2_sb[:, i, m * P : (m + 1) * P], t1[:, i * B : (i + 1) * B],
                        start=(i == 0), stop=(i == 3),
                    )
                nc.scalar.activation(
                    t2[:, m * B : (m + 1) * B], psg[:, :], AF.Gelu,
                    bias=ab2_sb[:, m : m + 1],
                )
            t3 = gw.tile([P, B], dt16, name="t3", tag="t3")
            psg = psgp.tile([P, B], dt32, name="psg3", tag="psg")
            for i in range(2):
                nc.tensor.matmul(
                    psg[:, :], a3_sb[:, i, :], t2[:, i * B : (i + 1) * B],
                    start=(i == 0), stop=(i == 1),
                )
            nc.scalar.activation(t3[:, :], psg[:, :], AF.Identity, bias=ab3_sb[:, 0:1])

            g1 = gw.tile([64, B], dt16, name="g1", tag="g1")
            psg = psgp.tile([64, B], dt32, name="psg4", tag="psg")
            nc.tensor.matmul(psg[:, :], s1_sb[:, :], t3[:, :], start=True, stop=True)
            nc.scalar.activation(g1[:, :], psg[:, :], AF.Gelu, bias=sb1_sb[:, 0:1])

            g2 = gw.tile([32, B], dt16, name="g2", tag="g2")
            psg = psgp.tile([32, B], dt32, name="psg5", tag="psg")
            nc.tensor.matmul(psg[:, :], s2_sb[:, :], g1[:, :], start=True, stop=True)
            nc.scalar.activation(g2[:, :], psg[:, :], AF.Gelu, bias=sb2_sb[:, 0:1])

            # flip to token-major: z[b, e]
            z = gw.tile([B, E], dt32, name="z", tag="z")
            psg = psgp.tile([B, E], dt32, name="psg6", tag="psg")
            nc.tensor.matmul(psg[:, :], g2[:, :], s3_sb[:, :], start=True, stop=True)
            nc.scalar.copy(z[:, :], psg[:, :])
            nc.vector.tensor_add(z[:, :], z[:, :], sb3_sb[:, :])

            # softmax over E (free dim)
            mx = gw.tile([B, 1], dt32, name="mx", tag="mx")
            nc.vector.reduce_max(mx[:, :], z[:, :], axis=AX.X)
            nc.vector.tensor_scalar_sub(z[:, :], z[:, :], mx[:, 0:1])
            sums = gw.tile([B, 1], dt32, name="sums", tag="sums")
            exps = gw.tile([B, E], dt32, name="exps", tag="exps")
            nc.scalar.activation(exps[:, :], z[:, :], AF.Exp, accum_out=sums[:, 0:1])
            rinv = gw.tile([B, 1], dt32, name="rinv", tag="rinv")
            nc.vector.reciprocal(rinv[:, :], sums[:, :])
            gate4 = gw.tile([B, E], dt32, name="gate4", tag="gate4")
            nc.vector.tensor_scalar_mul(gate4[:, :], exps[:, :], rinv[:, 0:1])

            # broadcast gate to all 128 partitions via DRAM bounce
            with tc.tile_pool(name="dramp", bufs=1, space="DRAM") as dp:
                gsc = dp.tile([1, B * E], dt32, name="gsc", tag="gsc")
                nc.sync.dma_start(
                    out=gsc.rearrange("o (b e) -> (o b) e", b=B), in_=gate4[:, :]
                )
                gflat = gw.tile([1, B * E], dt32, name="gflat", tag="gflat")
                nc.sync.dma_start(out=gflat[:, :], in_=gsc[:, :])
            nc.gpsimd.partition_broadcast(gate_bc[:, :], gflat[:, :])

            # gate-scaled e2 bias: e2bs[p, (i, b, e)] = e2bT[p, (i, e)] * gate[b, e]
            for i in range(HT):
                for b in range(B):
                    nc.vector.tensor_mul(
                        e2bs[:, i * (B * E) + b * E : i * (B * E) + (b + 1) * E],
                        e2bT_sb[:, i * E : (i + 1) * E],
                        gate_bc[:, b * E : (b + 1) * E],
                    )

        # ---------- tail constants (needed only at the end) ----------
        fbbc = pp.tile([P, H], dt32, name="fbbc", tag="fbbc")
        nc.sync.dma_start(out=fbbc[:, :], in_=t["fbbc_d"][:, :])
        gbc = pp.tile([P, H], dt32, name="gbc", tag="gbc")
        nc.sync.dma_start(out=gbc[:, :], in_=t["gbc_d"][:, :])
        bbc = pp.tile([P, H], dt32, name="bbc", tag="bbc")
        nc.sync.dma_start(out=bbc[:, :], in_=t["bbc_d"][:, :])

        # ---------- pools for the expert loop + interleaved tail ----------
        txf = tc.alloc_tile_pool(name="txf", bufs=5)
        ps3p = tc.alloc_tile_pool(name="ps3p", bufs=4, space="PSUM")
        tp = tc.alloc_tile_pool(name="tail", bufs=2)
        otp = tc.alloc_tile_pool(name="otp", bufs=2)
        sqp = tc.alloc_tile_pool(name="sqp", bufs=1)
        xrfs = {}

        def emit_xrf(tau):
            # residual + f_b prep; no expert-loop deps, runs on DVE slack
            xrf = txf.tile([P, H], dt32, name=f"xrf{tau}", tag="xrf")
            nc.sync.dma_start(out=xrf[:, :], in_=t["xres"][tau * P : (tau + 1) * P, :])
            nc.vector.tensor_add(xrf[:, :], xrf[:, :], fbbc[:, :])
            xrfs[tau] = xrf

        def emit_tail(tau):
            # layernorm tail for one token tile; DVE: +xrf, reduce, scale*g;
            # ACT: center, square-accum, sqrt; GpSimd: +ln_b.
            f2 = fused[tau]
            nc.vector.tensor_add(f2[:, :], f2[:, :], xrfs[tau][:, :])
            ssum = tp.tile([P, 1], dt32, name="ssum", tag="ssum")
            nc.vector.reduce_sum(ssum[:, :], f2[:, :], axis=AX.X)
            negmu = tp.tile([P, 1], dt32, name="negmu", tag="negmu")
            nc.vector.tensor_scalar_mul(negmu[:, :], ssum[:, :], -1.0 / H)
            nc.scalar.activation(f2[:, :], f2[:, :], AF.Identity, bias=negmu[:, 0:1])
            sq = sqp.tile([P, H], dt16, name="sq", tag="sq")
            ssq = tp.tile([P, 1], dt32, name="ssq", tag="ssq")
            nc.scalar.activation(sq[:, :], f2[:, :], AF.Square, accum_out=ssq[:, 0:1])
            stdv = tp.tile([P, 1], dt32, name="stdv", tag="stdv")
            nc.scalar.activation(
                stdv[:, :], ssq[:, :], AF.Sqrt, scale=1.0 / H, bias=eps[:, 0:1]
            )
            rinv2 = tp.tile([P, 1], dt32, name="rinv2", tag="rinv2")
            nc.vector.reciprocal(rinv2[:, :], stdv[:, :])
            ot = otp.tile([P, H], dt32, name="ot", tag="ot")
            nc.vector.scalar_tensor_tensor(
                ot[:, :], f2[:, :], rinv2[:, 0:1], gbc[:, :],
                op0=ALU.mult, op1=ALU.mult,
            )
            nc.gpsimd.tensor_add(ot[:, :], ot[:, :], bbc[:, :])
            nc.sync.dma_start(out=t["out"][tau * P : (tau + 1) * P, :], in_=ot[:, :])

        # ---------- main expert loop ----------
        for e in range(E):
            if e == E - 3:
                for tau in range(5):
                    emit_xrf(tau)
            w1t, w2t, wft = w_cache.pop(e) if e in w_cache else fetch_weights(e)
            h1t = hp.tile([P, F1T, TPC], dt16, name=f"h1t{e}", tag="h1t")
            eot = ep.tile([P, HT, TPC], dt16, name=f"eot{e}", tag="eot")

            for ch in range(NCH):
                c0, c1 = ch * TCH, (ch + 1) * TCH
                # mm1: h1 = gelu(x @ W1 + b1), feature-major
                for m in range(F1T):
                    ps = ps1p.tile([P, TCH], dt32, name="ps1", tag="ps1")
                    for i in range(HT):
                        nc.tensor.matmul(
                            ps[:, :], w1t[:, i, m * P : (m + 1) * P], xs[i][:, c0:c1],
                            start=(i == 0), stop=(i == HT - 1),
                        )
                    nc.scalar.activation(
                        h1t[:, m, c0:c1], ps[:, :], AF.Gelu,
                        bias=b1_sb[:, e * F1T + m : e * F1T + m + 1],
                    )
                # mm2: eo = (h1 @ W2 + b2) * gate, feature-major
                for i2 in range(HT):
                    ps = ps2p.tile([P, TCH], dt32, name="ps2", tag="ps2")
                    for m in range(F1T):
                        nc.tensor.matmul(
                            ps[:, :], w2t[:, m, i2 * P : (i2 + 1) * P], h1t[:, m, c0:c1],
                            start=(m == 0), stop=(m == F1T - 1),
                        )
                    for bh in range(2):
                        b = ch * 2 + bh
                        j = b * E + e
                        nc.scalar.activation(
                            eot[:, i2, b * SC : (b + 1) * SC],
                            ps[:, bh * SC : (bh + 1) * SC],
                            AF.Identity,
                            bias=e2bs[:, i2 * (B * E) + j : i2 * (B * E) + j + 1],
                            scale=gate_bc[:, j : j + 1],
                        )
                # mm3: fused += eo @ Wf, token-major out
                for tt_ in range(TCH // P):
                    tau = ch * (TCH // P) + tt_
                    for n in range(2):
                        ps = ps3p.tile([P, TCH], dt32, name="ps3", tag="ps3")
                        for i in range(HT):
                            nc.tensor.matmul(
                                ps[:, :],
                                eot[:, i, tau * P : (tau + 1) * P],
                                wft[:, i, n * TCH : (n + 1) * TCH],
                                start=(i == 0), stop=(i == HT - 1),
                            )
                        dst = fused[tau][:, n * TCH : (n + 1) * TCH]
                        if e == 0:
                            nc.vector.tensor_copy(dst, ps[:, :])
                        else:
                            nc.vector.tensor_add(dst, dst, ps[:, :])
                        if e == E - 1 and n == 1:
                            if tau + 3 >= 5 and tau + 3 < TT:
                                emit_xrf(tau + 3)
                            emit_tail(tau)
        sqp.release()
        otp.release()
        tp.release()
        ps3p.release()
        txf.release()


def _prep_inputs(inputs):
    """Host-side sharding/layout prep. Returns per-core input maps."""
    f32 = np.float32

    def bf(x):
        return np.ascontiguousarray(np.asarray(x, dtype=f32)).astype(BF16)

    hs = np.ascontiguousarray(np.asarray(inputs["hidden_states"], dtype=f32))  # [B,S,H]
    Xb = bf(hs.reshape(TOK, H))                                    # [8192, 1024] bf16

    e1_w = np.asarray(inputs["e1_w"], f32)
    e1_b = np.asarray(inputs["e1_b"], f32)
    e2_w = np.asarray(inputs["e2_w"], f32)
    e2_b = np.asarray(inputs["e2_b"], f32)
    f_w = np.asarray(inputs["f_w"], f32)

    common = {
        "w1": bf(e1_w).reshape(E, HT, P, F1),
        "w2": bf(e2_w).reshape(E, F1T, P, H),
        "wf": bf(f_w).reshape(E, HT, P, H),
        "b1d": np.ascontiguousarray(e1_b.reshape(E, F1T, P).transpose(2, 0, 1)).reshape(P, E * F1T),
        "e2bTd": np.ascontiguousarray(e2_b.reshape(E, HT, P).transpose(2, 1, 0)).reshape(P, HT * E),
        "a1": bf(inputs["a1_w"]).reshape(HT, P, F1),
        "a2": bf(inputs["a2_w"]).reshape(4, P, 256),
        "a3": bf(inputs["a3_w"]).reshape(2, P, 128),
        "s1": bf(inputs["s1_w"]),
        "s2": bf(inputs["s2_w"]),
        "s3": bf(inputs["s3_w"]),
        "ab1": np.ascontiguousarray(np.asarray(inputs["a1_b"], f32).reshape(4, P).T),
        "ab2": np.ascontiguousarray(np.asarray(inputs["a2_b"], f32).reshape(2, P).T),
        "ab3": np.ascontiguousarray(np.asarray(inputs["a3_b"], f32).reshape(1, P).T),
        "sb1": np.ascontiguousarray(np.asarray(inputs["s1_b"], f32).reshape(64, 1)),
        "sb2": np.ascontiguousarray(np.asarray(inputs["s2_b"], f32).reshape(32, 1)),
        "sb3": np.ascontiguousarray(np.broadcast_to(np.asarray(inputs["s3_b"], f32), (B, E))),
        "fbbc_d": np.ascontiguousarray(np.broadcast_to(np.asarray(inputs["f_b"], f32), (P, H))),
        "gbc_d": np.ascontiguousarray(np.broadcast_to(np.asarray(inputs["ln_g"], f32), (P, H))),
        "bbc_d": np.ascontiguousarray(np.broadcast_to(np.asarray(inputs["ln_b"], f32), (P, H))),
    }
    if not USE_COLLECTIVE:
        common["xpool"] = Xb
        common["ident4"] = np.eye(4, dtype=f32)

    hsb = Xb.reshape(B, S, H)
    in_maps = []
    for c in range(NCORES):
        shard16 = hsb[:, c * SC : (c + 1) * SC, :]                  # [B, SC, H] bf16
        xsh_c = np.ascontiguousarray(shard16.transpose(2, 0, 1)).reshape(H, TPC)
        xres_c = np.ascontiguousarray(hs[:, c * SC : (c + 1) * SC, :]).reshape(TPC, H)
        m = dict(common)
        m["xsh"] = xsh_c
        m["xres"] = xres_c
        in_maps.append(m)
    return in_maps


def kernel(**inputs) -> np.ndarray:
    nc = _build()
    in_maps = _prep_inputs(inputs)
    res = bass_utils.run_bass_kernel_spmd(nc, in_maps, core_ids=list(range(NCORES)))
    out_full = np.empty((B, S, H), dtype=np.float32)
    for c in range(NCORES):
        out_full[:, c * SC : (c + 1) * SC, :] = res.results[c]["out"].reshape(B, SC, H)
    return out_full
